# revision 1
# baseline (speedup 1.0000x reference)
"""Causal self-attention (B=4, T=2048, C=1024, H=16) on 8 TRN2 NeuronCores.

Sharding: core c handles batch b = c//2 and heads [8*(c%2), 8*(c%2)+8).
Each core computes the qkv projection for its 8 heads, flash-style causal
attention, and a partial output projection (its heads' slice of W_out rows).
Host sums the two partials per batch and adds the bias terms (v/out biases
are affine in the output because softmax rows sum to 1).

Schedule (single fused stream, tensor engine never drains):
  prefix   x arrives k-tile by k-tile; v(t0..3)/q(m0)/k(m0) projections run
           k-outer, pipelined against the DMA.
  steady   attention runs per (q-chunk, head-pair). The two heads of a pair
           occupy partitions 0:64 / 64:128, so their K=64 score matmuls
           land on disjoint PE row-groups and execute concurrently (row
           tiling). Remaining qkv projections and the output projection are
           chopped into single-matmul "fill" items, pumped between
           attention ops so the PE stays busy while ACT does the exps.
  denom    the ones-column in v makes the AV matmul emit the softmax
           denominator; 1/den via DVE reciprocal_approx_fast straight from
           PSUM, broadcast on gpsimd, one DVE multiply into aT.
All matmuls bf16 with fp32 PSUM accumulation; output y returned bf16
(partials are summed on host in fp32).
"""
from collections import deque

import numpy as np
import concourse.bass as bass  # noqa: F401  (registers engines)
import concourse.mybir as mybir
import concourse.tile as tile
from concourse import bacc
from concourse.bass_utils import run_bass_kernel_spmd

# problem constants (hardcoded per contract)
B, T, C, H, D = 4, 2048, 1024, 16, 64
NCORES = 8
NH = H // 2          # heads per core = 8
NHP = NH // 2        # head pairs per core = 4
QK = NH * D          # 512 qkv cols per core per q/k/v
SCALE = float(D) ** -0.5
P = 128
NKT = C // P         # 8 contraction tiles for the projections
NMQ = QK // P        # 4 row-tiles of qT/kT
NTT = T // P         # 16 t-blocks
NQC = T // 512       # 4 q-chunks
FP = mybir.dt.float32
BF = mybir.dt.bfloat16
EXP = mybir.ActivationFunctionType.Exp

PUMP = {0: 5, 1: 5, 2: 4, 3: 2}  # fill matmuls per attention pair-step

_NC_CACHE = {}
_LAST_IN_MAPS = None


def build_nc():
    if "nc" in _NC_CACHE:
        return _NC_CACHE["nc"]
    nc = bacc.Bacc(target_bir_lowering=False)
    xT = nc.declare_dram_parameter("xT", [C, T], BF, isOutput=False)
    Wq = nc.declare_dram_parameter("Wq", [NMQ, P, NKT * P], BF, isOutput=False)
    Wk = nc.declare_dram_parameter("Wk", [NMQ, P, NKT * P], BF, isOutput=False)
    Wv = nc.declare_dram_parameter("Wv", [P, NKT * QK], BF, isOutput=False)
    bq = nc.declare_dram_parameter("bq", [QK, 1], FP, isOutput=False)
    bk = nc.declare_dram_parameter("bk", [QK, 1], FP, isOutput=False)
    Wo = nc.declare_dram_parameter("Wo", [QK, C], BF, isOutput=False)
    tri = nc.declare_dram_parameter("tri", [P, P], BF, isOutput=False)
    y = nc.declare_dram_parameter("y", [T, C], BF, isOutput=True)

    with nc.allow_low_precision(reason="bf16 attention"), \
         tile.TileContext(nc) as tc, \
         tc.tile_pool(name="persist", bufs=1) as pers, \
         tc.tile_pool(name="psum", bufs=1, space="PSUM") as pp, \
         tc.tile_pool(name="evict", bufs=1) as ep:

        # ---- persistent tiles
        xsb = [pers.tile([P, T], BF, name=f"x{k}", tag=f"x{k}")
               for k in range(NKT)]
        qT = [pers.tile([P, T], BF, name=f"qT{m}", tag=f"qT{m}")
              for m in range(NMQ)]
        kT = [pers.tile([P, T], BF, name=f"kT{m}", tag=f"kT{m}")
              for m in range(NMQ)]
        aT = [pers.tile([P, T], BF, name=f"aT{m}", tag=f"aT{m}")
              for m in range(NMQ)]
        vsb = [pers.tile([P, NH * 65], BF, name=f"v{t}", tag=f"v{t}")
               for t in range(NTT)]
        wqt = [pers.tile([P, NKT * P], BF, name=f"wq{m}", tag=f"wq{m}")
               for m in range(NMQ)]
        wkt = [pers.tile([P, NKT * P], BF, name=f"wk{m}", tag=f"wk{m}")
               for m in range(NMQ)]
        wvt = pers.tile([P, NKT * QK // P * P], BF, name="wv", tag="wv")
        wot = [pers.tile([P, C], BF, name=f"wo{k}", tag=f"wo{k}")
               for k in range(NMQ)]
        trit = pers.tile([P, P], BF, name="trit", tag="trit")
        bqt = pers.tile([P, NMQ], FP, name="bqt", tag="bqt")
        bkt = pers.tile([P, NMQ], FP, name="bkt", tag="bkt")

        # ---- DMAs, in the order the prefix consumes them: wv/x chunk k
        # arrive together so the k-outer v matmuls start within ~2us.
        nc.sync.dma_start(trit, tri.ap())
        nc.sync.dma_start(bqt, bq.ap().rearrange("(m p) o -> p (m o)", p=P))
        nc.sync.dma_start(bkt, bk.ap().rearrange("(m p) o -> p (m o)", p=P))
        nc.sync.dma_start(wqt[0], Wq.ap()[0])
        nc.sync.dma_start(wkt[0], Wk.ap()[0])
        for k in range(NKT):
            nc.sync.dma_start(wvt[:, k * QK:(k + 1) * QK],
                              Wv.ap()[:, k * QK:(k + 1) * QK])
            nc.sync.dma_start(xsb[k], xT.ap()[k * P:(k + 1) * P, :])
        for m in range(1, NMQ):
            nc.sync.dma_start(wqt[m], Wq.ap()[m])
            nc.sync.dma_start(wkt[m], Wk.ap()[m])
        for k in range(NMQ):
            nc.sync.dma_start(wot[k], Wo.ap()[k * P:(k + 1) * P, :])

        def evict_v(tb, ps):
            vdst = vsb[tb].rearrange("p (g w) -> p g w", w=65)
            vsrc = ps.rearrange("p (g w) -> p g w", w=64)
            nc.vector.tensor_copy(vdst[:, :, 0:64], vsrc[:, :, :])
            nc.vector.memset(vdst[:, :, 64:65], 1.0)

        def evict_qk(dst, ps, bias, m, ch):
            nc.vector.tensor_scalar_add(
                dst[m][:, ch * 512:(ch + 1) * 512], ps, bias[:, m:m + 1])

        # ================= prefix: v(t0..3) + q/k(m0, cols 0:512) ========
        # v first, k-outer, paced by the x/wv chunk DMAs; q/k m0 after
        # (x is resident by then), so the in-order tensor queue never
        # blocks on a weight DMA that arrives late.
        pv = [pp.tile([P, 512], FP, name=f"pv{tb}",
                      tag="po" if tb < 2 else "fps", bufs=2)
              for tb in range(4)]
        pq0 = pp.tile([P, 1024], FP, name="pq0", tag="wide", bufs=2)
        pk0 = pp.tile([P, 1024], FP, name="pk0", tag="wide", bufs=2)
        # PE warmup: ~5us of matmuls on a zeroed tile while the first DMAs
        # land, so HAM un-throttles (1.2 -> 2.4 GHz) before real work.
        zt = ep.tile([P, 512], BF, name="warm", tag="warm", bufs=1)
        nc.vector.memset(zt, 0.0)
        for _ in range(24):
            nc.tensor.matmul(pq0[:, 0:512], zt[:, 0:P], zt,
                             start=True, stop=True)
        for k in range(NKT):
            st, sp = (k == 0), (k == NKT - 1)
            for tb in range(4):
                nc.tensor.matmul(
                    pv[tb], xsb[k][:, tb * P:(tb + 1) * P],
                    wvt[:, k * QK:(k + 1) * QK], start=st, stop=sp)
            nc.tensor.matmul(pq0[:, 0:512], wqt[0][:, k * P:(k + 1) * P],
                             xsb[k][:, 0:512], start=st, stop=sp)
            nc.tensor.matmul(pk0[:, 0:512], wkt[0][:, k * P:(k + 1) * P],
                             xsb[k][:, 0:512], start=st, stop=sp)
        for tb in range(4):
            evict_v(tb, pv[tb])
        evict_qk(qT, pq0[:, 0:512], bqt, 0, 0)
        evict_qk(kT, pk0[:, 0:512], bkt, 0, 0)

        # ================= fill-item machinery ===========================
        FQ = deque()

        def qk_chunk(which, m, ch):
            wt = wqt[m] if which == "q" else wkt[m]
            dst = qT if which == "q" else kT
            bias = bqt if which == "q" else bkt
            holder = []
            for k in range(NKT):
                def mm(k=k, holder=holder, wt=wt, m=m, ch=ch, which=which):
                    if k == 0:
                        holder.append(pp.tile(
                            [P, 512], FP, name=f"f{which}{m}{ch}",
                            tag="fps", bufs=2))
                    nc.tensor.matmul(
                        holder[0], wt[:, k * P:(k + 1) * P],
                        xsb[k][:, ch * 512:(ch + 1) * 512],
                        start=(k == 0), stop=(k == NKT - 1))
                FQ.append(mm)

            def ev(holder=holder, dst=dst, bias=bias, m=m, ch=ch):
                evict_qk(dst, holder[0], bias, m, ch)
            FQ.append(ev)

        def v_chunk(tb):
            holder = []
            for k in range(NKT):
                def mm(k=k, holder=holder, tb=tb):
                    if k == 0:
                        holder.append(pp.tile(
                            [P, 512], FP, name=f"fv{tb}", tag="fps", bufs=2))
                    nc.tensor.matmul(
                        holder[0], xsb[k][:, tb * P:(tb + 1) * P],
                        wvt[:, k * QK:(k + 1) * QK],
                        start=(k == 0), stop=(k == NKT - 1))
                FQ.append(mm)

            def ev(holder=holder, tb=tb):
                evict_v(tb, holder[0])
            FQ.append(ev)

        def outproj_tile(t):
            for n in range(2):
                holder = []
                for k in range(NMQ):
                    def mm(k=k, holder=holder, t=t, n=n):
                        if k == 0:
                            holder.append(pp.tile(
                                [P, 512], FP, name=f"fy{t}{n}",
                                tag="fps", bufs=2))
                        nc.tensor.matmul(
                            holder[0], aT[k][:, t * P:(t + 1) * P],
                            wot[k][:, n * 512:(n + 1) * 512],
                            start=(k == 0), stop=(k == NMQ - 1))
                    FQ.append(mm)

                def ev(holder=holder, t=t, n=n):
                    ye = ep.tile([P, 512], BF, name=f"ye{t}{n}", tag="ye",
                                 bufs=3)
                    nc.vector.tensor_copy(ye, holder[0])
                    nc.sync.dma_start(
                        y.ap()[t * P:(t + 1) * P, n * 512:(n + 1) * 512], ye)
                FQ.append(ev)

        def pump(n):
            while n > 0 and FQ:
                item = FQ.popleft()
                if callable(item):
                    item()
                    n -= 1

        def drain_to(tag):
            while FQ:
                item = FQ.popleft()
                if callable(item):
                    item()
                elif item == tag:
                    return

        # queue order = deadline order
        for m in range(1, NMQ):
            qk_chunk("q", m, 0)
            qk_chunk("k", m, 0)
            FQ.append(f"m{m}c0")
        for qc in range(1, NQC):
            for tb in range(4 * qc, 4 * qc + 4):
                v_chunk(tb)
            for m in range(NMQ):
                qk_chunk("q", m, qc)
                qk_chunk("k", m, qc)
            FQ.append(f"qc{qc}")

        # ================= attention =====================================
        def attention_pair(qc, hp, fpump=2):
            c0 = qc * 512
            vle, vlo = 65 * (2 * hp), 65 * (2 * hp + 1)
            po_e = pp.tile([P, 512], FP, name=f"poe{qc}{hp}", tag="po",
                           bufs=2)
            po_o = pp.tile([P, 512], FP, name=f"poo{qc}{hp}", tag="po",
                           bufs=2)
            jmax = 4 * qc + 3

            def flush_av(pend):
                spans, es_e, es_o = pend
                for j, lo, d, w in spans:
                    nc.tensor.matmul(
                        po_e[0:65, lo:lo + w], vsb[j][:, vle:vle + 65],
                        es_e[:, d:d + w],
                        start=(j == 0), stop=(j == jmax))
                for j, lo, d, w in spans:
                    nc.tensor.matmul(
                        po_o[0:65, lo:lo + w], vsb[j][:, vlo:vlo + 65],
                        es_o[:, d:d + w],
                        start=(j == 0), stop=(j == jmax))

            # two-deep software pipeline: AV consumes the exp from two
            # steps back, so the in-order tensor queue never parks on ACT.
            pends = deque()
            for pr in range(2 * qc + 2):
                spans = []
                dst = 0
                for i in range(2):
                    j = 2 * pr + i
                    r = j - 4 * qc
                    lo = 0 if r < 0 else 128 * r
                    w = 512 - lo
                    dst = max(dst, i * 512 if r < 1 else 0)
                    spans.append((j, lo, dst, w))
                    dst += w
                pump(2)
                if len(pends) >= 2:
                    flush_av(pends.popleft())
                pump(max(0, PUMP[qc] - 2))
                ps_e = pp.tile([P, 1024], FP, name=f"pse{qc}{hp}{pr}",
                               tag="wide", bufs=2)
                ps_o = pp.tile([P, 1024], FP, name=f"pso{qc}{hp}{pr}",
                               tag="wide", bufs=2)
                # row-tiled: even head on PE rows 0:64, odd on 64:128
                for j, lo, d, w in spans:
                    nc.tensor.matmul(
                        ps_e[:, d:d + w], kT[hp][0:64, j * P:(j + 1) * P],
                        qT[hp][0:64, c0 + lo:c0 + 512],
                        start=True, stop=True)
                es_e = ep.tile([P, 1024], BF, name=f"ese{qc}{hp}{pr}",
                               tag="es", bufs=8)
                es_o = ep.tile([P, 1024], BF, name=f"eso{qc}{hp}{pr}",
                               tag="es", bufs=8)
                e0 = spans[0][2]
                e1 = spans[1][2] + spans[1][3]
                nc.scalar.activation(es_e[:, e0:e1], ps_e[:, e0:e1], EXP,
                                     scale=SCALE)
                for j, lo, d, w in spans:
                    nc.tensor.matmul(
                        ps_o[:, d:d + w], kT[hp][64:128, j * P:(j + 1) * P],
                        qT[hp][64:128, c0 + lo:c0 + 512],
                        start=True, stop=True)
                nc.scalar.activation(es_o[:, e0:e1], ps_o[:, e0:e1], EXP,
                                     scale=SCALE)
                for j, lo, d, w in spans:
                    if j - 4 * qc >= 0:
                        nc.vector.tensor_mul(
                            es_e[:, d:d + 128], es_e[:, d:d + 128], trit)
                        nc.vector.tensor_mul(
                            es_o[:, d:d + 128], es_o[:, d:d + 128], trit)
                pends.append((spans, es_e, es_o))
            while pends:
                pump(fpump)
                flush_av(pends.popleft())

            # denominator row 64 -> 1/den -> broadcast -> normalize the
            # numerator straight out of PSUM into aT
            den_e = ep.tile([1, 512], FP, name=f"dne{qc}{hp}", tag="dn",
                            bufs=4)
            den_o = ep.tile([1, 512], FP, name=f"dno{qc}{hp}", tag="dn",
                            bufs=4)
            nc.vector.tensor_copy(den_e, po_e[64:65, :])
            nc.vector.tensor_copy(den_o, po_o[64:65, :])
            rs_e = ep.tile([1, 512], FP, name=f"rse{qc}{hp}", tag="rs",
                           bufs=4)
            rs_o = ep.tile([1, 512], FP, name=f"rso{qc}{hp}", tag="rs",
                           bufs=4)
            nc.vector.reciprocal_approx_fast(out=rs_e, in_=den_e)
            nc.vector.reciprocal_approx_fast(out=rs_o, in_=den_o)
            pump(2)
            bcs_e = ep.tile([64, 512], FP, name=f"bce{qc}{hp}", tag="bcs",
                            bufs=4)
            bcs_o = ep.tile([64, 512], FP, name=f"bco{qc}{hp}", tag="bcs",
                            bufs=4)
            nc.gpsimd.partition_broadcast(bcs_e, rs_e)
            nc.gpsimd.partition_broadcast(bcs_o, rs_o)
            nc.vector.tensor_mul(aT[hp][0:64, c0:c0 + 512], po_e[0:64, :],
                                 bcs_e)
            nc.vector.tensor_mul(aT[hp][64:128, c0:c0 + 512], po_o[0:64, :],
                                 bcs_o)

        for qc in range(NQC):
            if qc > 0:
                drain_to(f"qc{qc}")
            for hp in range(NHP):
                if qc == 0 and hp < NHP - 1:
                    # prefetch-drain the NEXT pair's q/k chunks so their
                    # DVE evictions complete during this pair's attention
                    drain_to(f"m{hp + 1}c0")
                attention_pair(qc, hp,
                               fpump=12 if (qc == 3 and hp == 3) else 2)
            for t in range(4 * qc, 4 * qc + 4):
                outproj_tile(t)

        drain_to(None)

    nc.compile()
    _NC_CACHE["nc"] = nc
    return nc


def kernel(x, W_qkv, b_qkv, W_out, b_out):
    global _LAST_IN_MAPS
    x = np.asarray(x, dtype=np.float32)
    W_qkv = np.asarray(W_qkv, dtype=np.float32)
    b_qkv = np.asarray(b_qkv, dtype=np.float32)
    W_out = np.asarray(W_out, dtype=np.float32)
    b_out = np.asarray(b_out, dtype=np.float32)
    import ml_dtypes

    bf16 = ml_dtypes.bfloat16
    tri = np.triu(np.ones((P, P), dtype=np.float32)).astype(bf16)
    in_maps = []
    for c in range(NCORES):
        b, hg = c // 2, c % 2
        cols = slice(hg * QK, (hg + 1) * QK)
        wq = W_qkv[:, 0 * C:1 * C][:, cols]
        wk = W_qkv[:, 1 * C:2 * C][:, cols]
        wv = W_qkv[:, 2 * C:3 * C][:, cols]
        in_maps.append({
            "xT": np.ascontiguousarray(x[b].T).astype(bf16),
            "Wq": np.ascontiguousarray(
                wq.reshape(NKT, P, NMQ, P).transpose(2, 1, 0, 3)
                .reshape(NMQ, P, NKT * P)).astype(bf16),
            "Wk": np.ascontiguousarray(
                wk.reshape(NKT, P, NMQ, P).transpose(2, 1, 0, 3)
                .reshape(NMQ, P, NKT * P)).astype(bf16),
            "Wv": np.ascontiguousarray(
                wv.reshape(NKT, P, QK).transpose(1, 0, 2)
                .reshape(P, NKT * QK)).astype(bf16),
            "bq": np.ascontiguousarray(b_qkv[0 * C:1 * C][cols, None]),
            "bk": np.ascontiguousarray(b_qkv[1 * C:2 * C][cols, None]),
            "Wo": np.ascontiguousarray(W_out[hg * QK:(hg + 1) * QK, :]).astype(bf16),
            "tri": tri,
        })
    _LAST_IN_MAPS = in_maps
    nc = build_nc()
    res = run_bass_kernel_spmd(nc, in_maps, core_ids=list(range(NCORES)))
    # v-bias and output bias are affine in the output: softmax rows sum to 1.
    extra = b_qkv[2 * C:3 * C] @ W_out + b_out
    out = np.empty((B, T, C), dtype=np.float32)
    for b in range(B):
        out[b] = (res.results[2 * b]["y"].astype(np.float32)
                  + res.results[2 * b + 1]["y"].astype(np.float32) + extra)
    return out



# revision 6
# speedup vs baseline: 1.0250x; 1.0250x over previous
"""Causal self-attention (B=4, T=2048, C=1024, H=16) on 8 TRN2 NeuronCores.

Sharding: core c handles batch b = c//2 and heads [8*(c%2), 8*(c%2)+8).
Each core computes the qkv projection for its 8 heads, flash-style causal
attention, and a partial output projection (its heads' slice of W_out rows).
Host sums the two partials per batch and adds the bias terms (v/out biases
are affine in the output because softmax rows sum to 1).

Schedule (single fused stream, tensor engine never drains):
  prefix   x arrives k-tile by k-tile; v(t0..3)/q(m0)/k(m0) projections run
           k-outer, pipelined against the DMA.
  steady   attention runs per (q-chunk, head-pair). The two heads of a pair
           occupy partitions 0:64 / 64:128, so their K=64 score matmuls
           land on disjoint PE row-groups and execute concurrently (row
           tiling). Remaining qkv projections and the output projection are
           chopped into single-matmul "fill" items, pumped between
           attention ops so the PE stays busy while ACT does the exps.
  denom    the ones-column in v makes the AV matmul emit the softmax
           denominator; 1/den via DVE reciprocal_approx_fast straight from
           PSUM, broadcast on gpsimd, one DVE multiply into aT.
All matmuls bf16 with fp32 PSUM accumulation; output y returned bf16
(partials are summed on host in fp32).
"""
from collections import deque

import numpy as np
import concourse.bass as bass  # noqa: F401  (registers engines)
import concourse.mybir as mybir
import concourse.tile as tile
from concourse import bacc
from concourse.bass_utils import run_bass_kernel_spmd

# problem constants (hardcoded per contract)
B, T, C, H, D = 4, 2048, 1024, 16, 64
NCORES = 8
NH = H // 2          # heads per core = 8
NHP = NH // 2        # head pairs per core = 4
QK = NH * D          # 512 qkv cols per core per q/k/v
SCALE = float(D) ** -0.5
P = 128
NKT = C // P         # 8 contraction tiles for the projections
NMQ = QK // P        # 4 row-tiles of qT/kT
NTT = T // P         # 16 t-blocks
NQC = T // 512       # 4 q-chunks
FP = mybir.dt.float32
BF = mybir.dt.bfloat16
EXP = mybir.ActivationFunctionType.Exp

PUMP = {0: 6, 1: 6, 2: 5, 3: 4}  # fill matmuls per attention pair-step

_NC_CACHE = {}
_LAST_IN_MAPS = None


def build_nc():
    if "nc" in _NC_CACHE:
        return _NC_CACHE["nc"]
    nc = bacc.Bacc(target_bir_lowering=False)
    xT = nc.declare_dram_parameter("xT", [C, T], BF, isOutput=False)
    Wq = nc.declare_dram_parameter("Wq", [NMQ, P, NKT * P], BF, isOutput=False)
    Wk = nc.declare_dram_parameter("Wk", [NMQ, P, NKT * P], BF, isOutput=False)
    Wv = nc.declare_dram_parameter("Wv", [P, NKT * QK], BF, isOutput=False)
    bq = nc.declare_dram_parameter("bq", [QK, 1], FP, isOutput=False)
    bk = nc.declare_dram_parameter("bk", [QK, 1], FP, isOutput=False)
    Wo = nc.declare_dram_parameter("Wo", [QK, C], BF, isOutput=False)
    tri = nc.declare_dram_parameter("tri", [P, P], BF, isOutput=False)
    y = nc.declare_dram_parameter("y", [T, C], BF, isOutput=True)

    with nc.allow_low_precision(reason="bf16 attention"), \
         tile.TileContext(nc) as tc, \
         tc.tile_pool(name="persist", bufs=1) as pers, \
         tc.tile_pool(name="psum", bufs=1, space="PSUM") as pp, \
         tc.tile_pool(name="evict", bufs=1) as ep:

        # ---- persistent tiles
        xsb = [pers.tile([P, T], BF, name=f"x{k}", tag=f"x{k}")
               for k in range(NKT)]
        qT = [pers.tile([P, T], BF, name=f"qT{m}", tag=f"qT{m}")
              for m in range(NMQ)]
        kT = [pers.tile([P, T], BF, name=f"kT{m}", tag=f"kT{m}")
              for m in range(NMQ)]
        aT = [pers.tile([P, T], BF, name=f"aT{m}", tag=f"aT{m}")
              for m in range(NMQ)]
        vsb = [pers.tile([P, NH * 65], BF, name=f"v{t}", tag=f"v{t}")
               for t in range(NTT)]
        wqt = [pers.tile([P, NKT * P], BF, name=f"wq{m}", tag=f"wq{m}")
               for m in range(NMQ)]
        wkt = [pers.tile([P, NKT * P], BF, name=f"wk{m}", tag=f"wk{m}")
               for m in range(NMQ)]
        wvt = pers.tile([P, NKT * QK // P * P], BF, name="wv", tag="wv")
        wot = [pers.tile([P, C], BF, name=f"wo{k}", tag=f"wo{k}")
               for k in range(NMQ)]
        trit = pers.tile([P, P], BF, name="trit", tag="trit")
        bqt = pers.tile([P, NMQ], FP, name="bqt", tag="bqt")
        bkt = pers.tile([P, NMQ], FP, name="bkt", tag="bkt")

        # ---- DMAs, in the order the prefix consumes them: wv/x chunk k
        # arrive together so the k-outer v matmuls start within ~2us.
        nc.sync.dma_start(trit, tri.ap())
        nc.sync.dma_start(bqt, bq.ap().rearrange("(m p) o -> p (m o)", p=P))
        nc.sync.dma_start(bkt, bk.ap().rearrange("(m p) o -> p (m o)", p=P))
        nc.sync.dma_start(wqt[0], Wq.ap()[0])
        nc.sync.dma_start(wkt[0], Wk.ap()[0])
        for k in range(NKT):
            nc.sync.dma_start(wvt[:, k * QK:(k + 1) * QK],
                              Wv.ap()[:, k * QK:(k + 1) * QK])
            nc.sync.dma_start(xsb[k], xT.ap()[k * P:(k + 1) * P, :])
        for m in range(1, NMQ):
            nc.sync.dma_start(wqt[m], Wq.ap()[m])
            nc.sync.dma_start(wkt[m], Wk.ap()[m])
        for k in range(NMQ):
            nc.sync.dma_start(wot[k], Wo.ap()[k * P:(k + 1) * P, :])

        def evict_v(tb, ps):
            vdst = vsb[tb].rearrange("p (g w) -> p g w", w=65)
            vsrc = ps.rearrange("p (g w) -> p g w", w=64)
            nc.vector.tensor_copy(vdst[:, :, 0:64], vsrc[:, :, :])
            nc.vector.memset(vdst[:, :, 64:65], 1.0)

        def evict_qk(dst, ps, bias, m, ch):
            nc.vector.tensor_scalar_add(
                dst[m][:, ch * 512:(ch + 1) * 512], ps, bias[:, m:m + 1])

        # ================= prefix: v(t0..3) + q/k(m0, cols 0:512) ========
        # v first, k-outer, paced by the x/wv chunk DMAs; q/k m0 after
        # (x is resident by then), so the in-order tensor queue never
        # blocks on a weight DMA that arrives late.
        pv = [pp.tile([P, 512], FP, name=f"pv{tb}",
                      tag="po" if tb < 2 else "fps", bufs=2)
              for tb in range(4)]
        pq0 = pp.tile([P, 1024], FP, name="pq0", tag="wide", bufs=2)
        pk0 = pp.tile([P, 1024], FP, name="pk0", tag="wide", bufs=2)
        # PE warmup: ~5us of matmuls on a zeroed tile while the first DMAs
        # land, so HAM un-throttles (1.2 -> 2.4 GHz) before real work.
        zt = ep.tile([P, 512], BF, name="warm", tag="warm", bufs=1)
        nc.vector.memset(zt, 0.0)
        for _ in range(9):
            nc.tensor.matmul(pq0[:, 0:512], zt[:, 0:P], zt,
                             start=True, stop=True)
        for k in range(NKT):
            st, sp = (k == 0), (k == NKT - 1)
            for tb in range(4):
                nc.tensor.matmul(
                    pv[tb], xsb[k][:, tb * P:(tb + 1) * P],
                    wvt[:, k * QK:(k + 1) * QK], start=st, stop=sp)
            nc.tensor.matmul(pq0[:, 0:512], wqt[0][:, k * P:(k + 1) * P],
                             xsb[k][:, 0:512], start=st, stop=sp)
            nc.tensor.matmul(pk0[:, 0:512], wkt[0][:, k * P:(k + 1) * P],
                             xsb[k][:, 0:512], start=st, stop=sp)
        for tb in range(4):
            evict_v(tb, pv[tb])
        evict_qk(qT, pq0[:, 0:512], bqt, 0, 0)
        evict_qk(kT, pk0[:, 0:512], bkt, 0, 0)

        # ================= fill-item machinery ===========================
        FQ = deque()

        def qk_chunk(which, m, ch):
            wt = wqt[m] if which == "q" else wkt[m]
            dst = qT if which == "q" else kT
            bias = bqt if which == "q" else bkt
            holder = []
            for k in range(NKT):
                def mm(k=k, holder=holder, wt=wt, m=m, ch=ch, which=which):
                    if k == 0:
                        holder.append(pp.tile(
                            [P, 512], FP, name=f"f{which}{m}{ch}",
                            tag="fps", bufs=2))
                    nc.tensor.matmul(
                        holder[0], wt[:, k * P:(k + 1) * P],
                        xsb[k][:, ch * 512:(ch + 1) * 512],
                        start=(k == 0), stop=(k == NKT - 1))
                FQ.append(mm)

            def ev(holder=holder, dst=dst, bias=bias, m=m, ch=ch):
                evict_qk(dst, holder[0], bias, m, ch)
            FQ.append(ev)

        def v_chunk(tb):
            holder = []
            for k in range(NKT):
                def mm(k=k, holder=holder, tb=tb):
                    if k == 0:
                        holder.append(pp.tile(
                            [P, 512], FP, name=f"fv{tb}", tag="fps", bufs=2))
                    nc.tensor.matmul(
                        holder[0], xsb[k][:, tb * P:(tb + 1) * P],
                        wvt[:, k * QK:(k + 1) * QK],
                        start=(k == 0), stop=(k == NKT - 1))
                FQ.append(mm)

            def ev(holder=holder, tb=tb):
                evict_v(tb, holder[0])
            FQ.append(ev)

        def outproj_tile(t):
            for n in range(2):
                holder = []
                for k in range(NMQ):
                    def mm(k=k, holder=holder, t=t, n=n):
                        if k == 0:
                            holder.append(pp.tile(
                                [P, 512], FP, name=f"fy{t}{n}",
                                tag="fps", bufs=2))
                        nc.tensor.matmul(
                            holder[0], aT[k][:, t * P:(t + 1) * P],
                            wot[k][:, n * 512:(n + 1) * 512],
                            start=(k == 0), stop=(k == NMQ - 1))
                    FQ.append(mm)

                def ev(holder=holder, t=t, n=n):
                    ye = ep.tile([P, 512], BF, name=f"ye{t}{n}", tag="ye",
                                 bufs=3)
                    nc.vector.tensor_copy(ye, holder[0])
                    nc.sync.dma_start(
                        y.ap()[t * P:(t + 1) * P, n * 512:(n + 1) * 512], ye)
                FQ.append(ev)

        def pump(n):
            while n > 0 and FQ:
                item = FQ.popleft()
                if callable(item):
                    item()
                    n -= 1

        def drain_to(tag):
            while FQ:
                item = FQ.popleft()
                if callable(item):
                    item()
                elif item == tag:
                    return

        # queue order = deadline order
        for m in range(1, NMQ):
            qk_chunk("q", m, 0)
            qk_chunk("k", m, 0)
            FQ.append(f"m{m}c0")
        for qc in range(1, NQC):
            for tb in range(4 * qc, 4 * qc + 4):
                v_chunk(tb)
            for m in range(NMQ):
                qk_chunk("q", m, qc)
                qk_chunk("k", m, qc)
            FQ.append(f"qc{qc}")

        # ================= attention =====================================
        def attention_pair(qc, hp, fpump=2):
            c0 = qc * 512
            vle, vlo = 65 * (2 * hp), 65 * (2 * hp + 1)
            po_e = pp.tile([P, 512], FP, name=f"poe{qc}{hp}", tag="po",
                           bufs=2)
            po_o = pp.tile([P, 512], FP, name=f"poo{qc}{hp}", tag="po",
                           bufs=2)
            jmax = 4 * qc + 3

            def flush_av(pend):
                spans, es_e, es_o = pend
                for j, lo, d, w in spans:
                    nc.tensor.matmul(
                        po_e[0:65, lo:lo + w], vsb[j][:, vle:vle + 65],
                        es_e[:, d:d + w],
                        start=(j == 0), stop=(j == jmax))
                for j, lo, d, w in spans:
                    nc.tensor.matmul(
                        po_o[0:65, lo:lo + w], vsb[j][:, vlo:vlo + 65],
                        es_o[:, d:d + w],
                        start=(j == 0), stop=(j == jmax))

            # two-deep software pipeline: AV consumes the exp from two
            # steps back, so the in-order tensor queue never parks on ACT.
            pends = deque()
            for pr in range(2 * qc + 2):
                spans = []
                dst = 0
                for i in range(2):
                    j = 2 * pr + i
                    r = j - 4 * qc
                    lo = 0 if r < 0 else 128 * r
                    w = 512 - lo
                    dst = max(dst, i * 512 if r < 1 else 0)
                    spans.append((j, lo, dst, w))
                    dst += w
                pump(2)
                if len(pends) >= 2:
                    flush_av(pends.popleft())
                pump(max(0, PUMP[qc] - 2))
                ps_e = pp.tile([P, 1024], FP, name=f"pse{qc}{hp}{pr}",
                               tag="wide", bufs=2)
                ps_o = pp.tile([P, 1024], FP, name=f"pso{qc}{hp}{pr}",
                               tag="wide", bufs=2)
                # row-tiled: even head on PE rows 0:64, odd on 64:128.
                # Interleave e/o per span so consecutive matmuls sit on
                # different row groups: the odd ldweights pulls ahead and
                # both streams run concurrently on the PE.
                for j, lo, d, w in spans:
                    nc.tensor.matmul(
                        ps_e[:, d:d + w], kT[hp][0:64, j * P:(j + 1) * P],
                        qT[hp][0:64, c0 + lo:c0 + 512],
                        start=True, stop=True)
                    nc.tensor.matmul(
                        ps_o[:, d:d + w], kT[hp][64:128, j * P:(j + 1) * P],
                        qT[hp][64:128, c0 + lo:c0 + 512],
                        start=True, stop=True)
                es_e = ep.tile([P, 1024], BF, name=f"ese{qc}{hp}{pr}",
                               tag="es", bufs=8)
                es_o = ep.tile([P, 1024], BF, name=f"eso{qc}{hp}{pr}",
                               tag="es", bufs=8)
                e0 = spans[0][2]
                e1 = spans[1][2] + spans[1][3]
                nc.scalar.activation(es_e[:, e0:e1], ps_e[:, e0:e1], EXP,
                                     scale=SCALE)
                nc.scalar.activation(es_o[:, e0:e1], ps_o[:, e0:e1], EXP,
                                     scale=SCALE)
                for j, lo, d, w in spans:
                    if j - 4 * qc >= 0:
                        nc.vector.tensor_mul(
                            es_e[:, d:d + 128], es_e[:, d:d + 128], trit)
                        nc.vector.tensor_mul(
                            es_o[:, d:d + 128], es_o[:, d:d + 128], trit)
                pends.append((spans, es_e, es_o))
            while pends:
                pump(fpump)
                flush_av(pends.popleft())

            # denominator row 64 -> 1/den -> broadcast -> normalize the
            # numerator straight out of PSUM into aT. Fill matmuls are
            # pumped between the steps so the PE never idles under this
            # latency chain (it is fully exposed on the final pair).
            den_e = ep.tile([1, 512], FP, name=f"dne{qc}{hp}", tag="dn",
                            bufs=4)
            den_o = ep.tile([1, 512], FP, name=f"dno{qc}{hp}", tag="dn",
                            bufs=4)
            nc.vector.tensor_copy(den_e, po_e[64:65, :])
            nc.vector.tensor_copy(den_o, po_o[64:65, :])
            pump(2)
            rs = ep.tile([1, 1024], FP, name=f"rs{qc}{hp}", tag="rs",
                         bufs=4)
            nc.vector.reciprocal_approx_fast(out=rs[:, 0:512], in_=den_e)
            nc.vector.reciprocal_approx_fast(out=rs[:, 512:1024], in_=den_o)
            pump(2)
            bcs = ep.tile([64, 1024], FP, name=f"bc{qc}{hp}", tag="bcs",
                          bufs=4)
            nc.gpsimd.partition_broadcast(bcs, rs)
            pump(2)
            nc.vector.tensor_mul(aT[hp][0:64, c0:c0 + 512], po_e[0:64, :],
                                 bcs[:, 0:512])
            nc.vector.tensor_mul(aT[hp][64:128, c0:c0 + 512], po_o[0:64, :],
                                 bcs[:, 512:1024])

        for qc in range(NQC):
            if qc > 0:
                drain_to(f"qc{qc}")
            for hp in range(NHP):
                if qc == 0 and hp < NHP - 1:
                    # prefetch-drain the NEXT pair's q/k chunks so their
                    # DVE evictions complete during this pair's attention
                    drain_to(f"m{hp + 1}c0")
                attention_pair(qc, hp, fpump=2)
            for t in range(4 * qc, 4 * qc + 4):
                outproj_tile(t)

        drain_to(None)

    nc.compile()
    _NC_CACHE["nc"] = nc
    return nc


def kernel(x, W_qkv, b_qkv, W_out, b_out):
    global _LAST_IN_MAPS
    x = np.asarray(x, dtype=np.float32)
    W_qkv = np.asarray(W_qkv, dtype=np.float32)
    b_qkv = np.asarray(b_qkv, dtype=np.float32)
    W_out = np.asarray(W_out, dtype=np.float32)
    b_out = np.asarray(b_out, dtype=np.float32)
    import ml_dtypes

    bf16 = ml_dtypes.bfloat16
    tri = np.triu(np.ones((P, P), dtype=np.float32)).astype(bf16)
    in_maps = []
    for c in range(NCORES):
        b, hg = c // 2, c % 2
        cols = slice(hg * QK, (hg + 1) * QK)
        wq = W_qkv[:, 0 * C:1 * C][:, cols]
        wk = W_qkv[:, 1 * C:2 * C][:, cols]
        wv = W_qkv[:, 2 * C:3 * C][:, cols]
        in_maps.append({
            "xT": np.ascontiguousarray(x[b].T).astype(bf16),
            "Wq": np.ascontiguousarray(
                wq.reshape(NKT, P, NMQ, P).transpose(2, 1, 0, 3)
                .reshape(NMQ, P, NKT * P)).astype(bf16),
            "Wk": np.ascontiguousarray(
                wk.reshape(NKT, P, NMQ, P).transpose(2, 1, 0, 3)
                .reshape(NMQ, P, NKT * P)).astype(bf16),
            "Wv": np.ascontiguousarray(
                wv.reshape(NKT, P, QK).transpose(1, 0, 2)
                .reshape(P, NKT * QK)).astype(bf16),
            "bq": np.ascontiguousarray(b_qkv[0 * C:1 * C][cols, None]),
            "bk": np.ascontiguousarray(b_qkv[1 * C:2 * C][cols, None]),
            "Wo": np.ascontiguousarray(W_out[hg * QK:(hg + 1) * QK, :]).astype(bf16),
            "tri": tri,
        })
    _LAST_IN_MAPS = in_maps
    nc = build_nc()
    res = run_bass_kernel_spmd(nc, in_maps, core_ids=list(range(NCORES)))
    # v-bias and output bias are affine in the output: softmax rows sum to 1.
    extra = b_qkv[2 * C:3 * C] @ W_out + b_out
    out = np.empty((B, T, C), dtype=np.float32)
    for b in range(B):
        out[b] = (res.results[2 * b]["y"].astype(np.float32)
                  + res.results[2 * b + 1]["y"].astype(np.float32) + extra)
    return out



# revision 11
# speedup vs baseline: 1.1305x; 1.1030x over previous
"""Causal self-attention (B=4, T=2048, C=1024, H=16) on 8 TRN2 NeuronCores.

Sharding: core c handles batch b = c//2 and heads [8*(c%2), 8*(c%2)+8).
Each core computes the qkv projection for its 8 heads, flash-style causal
attention, and a partial output projection (its heads' slice of W_out rows).
Host sums the two partials per batch and adds the bias terms (v/out biases
are affine in the output because softmax rows sum to 1).

Schedule (single fused stream, tensor engine never drains):
  prefix   x arrives k-tile by k-tile; v(t0..3)/q(m0)/k(m0) projections run
           k-outer, pipelined against the DMA.
  steady   attention runs per (q-chunk, head-pair). The two heads of a pair
           occupy partitions 0:64 / 64:128, so their K=64 score matmuls
           land on disjoint PE row-groups and execute concurrently (row
           tiling). Remaining qkv projections and the output projection are
           chopped into single-matmul "fill" items, pumped between
           attention ops so the PE stays busy while ACT does the exps.
  denom    the ones-column in v makes the AV matmul emit the softmax
           denominator; 1/den via DVE reciprocal_approx_fast straight from
           PSUM, broadcast on gpsimd, one DVE multiply into aT.
All matmuls bf16 with fp32 PSUM accumulation; output y returned bf16
(partials are summed on host in fp32).
"""
from collections import deque

import numpy as np
import concourse.bass as bass  # noqa: F401  (registers engines)
import concourse.mybir as mybir
import concourse.tile as tile
from concourse import bacc
from concourse.bass_utils import run_bass_kernel_spmd

# problem constants (hardcoded per contract)
B, T, C, H, D = 4, 2048, 1024, 16, 64
NCORES = 8
NH = H // 2          # heads per core = 8
NHP = NH // 2        # head pairs per core = 4
QK = NH * D          # 512 qkv cols per core per q/k/v
SCALE = float(D) ** -0.5
P = 128
NKT = C // P         # 8 contraction tiles for the projections
NMQ = QK // P        # 4 row-tiles of qT/kT
NTT = T // P         # 16 t-blocks
NQC = T // 512       # 4 q-chunks
FP = mybir.dt.float32
BF = mybir.dt.bfloat16
EXP = mybir.ActivationFunctionType.Exp

PUMP = {0: 3, 1: 3, 2: 3, 3: 3}  # fill matmuls per attention pair-step

_NC_CACHE = {}
_LAST_IN_MAPS = None


def build_nc():
    if "nc" in _NC_CACHE:
        return _NC_CACHE["nc"]
    nc = bacc.Bacc(target_bir_lowering=False)
    xT = nc.declare_dram_parameter("xT", [C, T], BF, isOutput=False)
    Wq = nc.declare_dram_parameter("Wq", [NMQ, P, NKT * P], BF, isOutput=False)
    Wk = nc.declare_dram_parameter("Wk", [NMQ, P, NKT * P], BF, isOutput=False)
    Wv = nc.declare_dram_parameter("Wv", [P, NKT * QK], BF, isOutput=False)
    bq = nc.declare_dram_parameter("bq", [QK, 1], FP, isOutput=False)
    bk = nc.declare_dram_parameter("bk", [QK, 1], FP, isOutput=False)
    Wo = nc.declare_dram_parameter("Wo", [QK, C], BF, isOutput=False)
    tri = nc.declare_dram_parameter("tri", [P, P], BF, isOutput=False)
    y = nc.declare_dram_parameter("y", [T, C], BF, isOutput=True)

    with nc.allow_low_precision(reason="bf16 attention"), \
         tile.TileContext(nc) as tc, \
         tc.tile_pool(name="persist", bufs=1) as pers, \
         tc.tile_pool(name="psum", bufs=1, space="PSUM") as pp, \
         tc.tile_pool(name="evict", bufs=1) as ep:

        # ---- persistent tiles
        xsb = [pers.tile([P, T], BF, name=f"x{k}", tag=f"x{k}")
               for k in range(NKT)]
        qT = [pers.tile([P, T], BF, name=f"qT{m}", tag=f"qT{m}")
              for m in range(NMQ)]
        kT = [pers.tile([P, T], BF, name=f"kT{m}", tag=f"kT{m}")
              for m in range(NMQ)]
        aT = [pers.tile([P, T], BF, name=f"aT{m}", tag=f"aT{m}")
              for m in range(NMQ)]
        vsb = [pers.tile([P, NH * 65], BF, name=f"v{t}", tag=f"v{t}")
               for t in range(NTT)]
        wqt = [pers.tile([P, NKT * P], BF, name=f"wq{m}", tag=f"wq{m}")
               for m in range(NMQ)]
        wkt = [pers.tile([P, NKT * P], BF, name=f"wk{m}", tag=f"wk{m}")
               for m in range(NMQ)]
        wvt = pers.tile([P, NKT * QK // P * P], BF, name="wv", tag="wv")
        wot = [pers.tile([P, C], BF, name=f"wo{k}", tag=f"wo{k}")
               for k in range(NMQ)]
        trit = pers.tile([P, P], BF, name="trit", tag="trit")
        bqt = pers.tile([P, NMQ], FP, name="bqt", tag="bqt")
        bkt = pers.tile([P, NMQ], FP, name="bkt", tag="bkt")

        # ---- DMAs, in the order the prefix consumes them: wv/x chunk k
        # arrive together so the k-outer v matmuls start within ~2us.
        nc.sync.dma_start(trit, tri.ap())
        nc.sync.dma_start(bqt, bq.ap().rearrange("(m p) o -> p (m o)", p=P))
        nc.sync.dma_start(bkt, bk.ap().rearrange("(m p) o -> p (m o)", p=P))
        nc.sync.dma_start(wqt[0], Wq.ap()[0])
        nc.sync.dma_start(wkt[0], Wk.ap()[0])
        for k in range(NKT):
            nc.sync.dma_start(wvt[:, k * QK:(k + 1) * QK],
                              Wv.ap()[:, k * QK:(k + 1) * QK])
            nc.sync.dma_start(xsb[k], xT.ap()[k * P:(k + 1) * P, :])
        for m in range(1, NMQ):
            nc.sync.dma_start(wqt[m], Wq.ap()[m])
            nc.sync.dma_start(wkt[m], Wk.ap()[m])
        for k in range(NMQ):
            nc.sync.dma_start(wot[k], Wo.ap()[k * P:(k + 1) * P, :])

        def evict_v(tb, ps):
            vdst = vsb[tb].rearrange("p (g w) -> p g w", w=65)
            vsrc = ps.rearrange("p (g w) -> p g w", w=64)
            nc.vector.tensor_copy(vdst[:, :, 0:64], vsrc[:, :, :])
            nc.vector.memset(vdst[:, :, 64:65], 1.0)

        def evict_qk(dst, ps, bias, m, ch):
            nc.vector.tensor_scalar_add(
                dst[m][:, ch * 512:(ch + 1) * 512], ps, bias[:, m:m + 1])

        # ================= prefix: v(t0..3) + q/k(m0, cols 0:512) ========
        # v first, k-outer, paced by the x/wv chunk DMAs; q/k m0 after
        # (x is resident by then), so the in-order tensor queue never
        # blocks on a weight DMA that arrives late.
        pv = [pp.tile([P, 512], FP, name=f"pv{tb}",
                      tag="po" if tb < 2 else "fps", bufs=2)
              for tb in range(4)]
        pq0 = pp.tile([P, 1024], FP, name="pq0", tag="wide", bufs=2)
        pk0 = pp.tile([P, 1024], FP, name="pk0", tag="wide", bufs=2)
        # PE warmup: ~5us of matmuls on a zeroed tile while the first DMAs
        # land, so HAM un-throttles (1.2 -> 2.4 GHz) before real work.
        zt = ep.tile([P, 512], BF, name="warm", tag="warm", bufs=1)
        nc.vector.memset(zt, 0.0)
        for _ in range(24):
            nc.tensor.matmul(pq0[:, 0:512], zt[:, 0:P], zt,
                             start=True, stop=True)
        for k in range(NKT):
            st, sp = (k == 0), (k == NKT - 1)
            for tb in range(4):
                nc.tensor.matmul(
                    pv[tb], xsb[k][:, tb * P:(tb + 1) * P],
                    wvt[:, k * QK:(k + 1) * QK], start=st, stop=sp)
            nc.tensor.matmul(pq0[:, 0:512], wqt[0][:, k * P:(k + 1) * P],
                             xsb[k][:, 0:512], start=st, stop=sp)
            nc.tensor.matmul(pk0[:, 0:512], wkt[0][:, k * P:(k + 1) * P],
                             xsb[k][:, 0:512], start=st, stop=sp)
        for tb in range(4):
            evict_v(tb, pv[tb])
        evict_qk(qT, pq0[:, 0:512], bqt, 0, 0)
        evict_qk(kT, pk0[:, 0:512], bkt, 0, 0)

        # ================= fill-item machinery ===========================
        FQ = deque()

        def qk_chunk(which, m, ch):
            wt = wqt[m] if which == "q" else wkt[m]
            dst = qT if which == "q" else kT
            bias = bqt if which == "q" else bkt
            holder = []
            for k in range(NKT):
                def mm(k=k, holder=holder, wt=wt, m=m, ch=ch, which=which):
                    if k == 0:
                        holder.append(pp.tile(
                            [P, 512], FP, name=f"f{which}{m}{ch}",
                            tag="fps", bufs=2))
                    nc.tensor.matmul(
                        holder[0], wt[:, k * P:(k + 1) * P],
                        xsb[k][:, ch * 512:(ch + 1) * 512],
                        start=(k == 0), stop=(k == NKT - 1))
                FQ.append(mm)

            def ev(holder=holder, dst=dst, bias=bias, m=m, ch=ch):
                evict_qk(dst, holder[0], bias, m, ch)
            FQ.append(ev)

        def v_chunk(tb):
            holder = []
            for k in range(NKT):
                def mm(k=k, holder=holder, tb=tb):
                    if k == 0:
                        holder.append(pp.tile(
                            [P, 512], FP, name=f"fv{tb}", tag="fps", bufs=2))
                    nc.tensor.matmul(
                        holder[0], xsb[k][:, tb * P:(tb + 1) * P],
                        wvt[:, k * QK:(k + 1) * QK],
                        start=(k == 0), stop=(k == NKT - 1))
                FQ.append(mm)

            def ev(holder=holder, tb=tb):
                evict_v(tb, holder[0])
            FQ.append(ev)

        def outproj_tile(t):
            for n in range(2):
                holder = []
                for k in range(NMQ):
                    def mm(k=k, holder=holder, t=t, n=n):
                        if k == 0:
                            holder.append(pp.tile(
                                [P, 512], FP, name=f"fy{t}{n}",
                                tag="fps", bufs=2))
                        nc.tensor.matmul(
                            holder[0], aT[k][:, t * P:(t + 1) * P],
                            wot[k][:, n * 512:(n + 1) * 512],
                            start=(k == 0), stop=(k == NMQ - 1))
                    FQ.append(mm)

                def ev(holder=holder, t=t, n=n):
                    ye = ep.tile([P, 512], BF, name=f"ye{t}{n}", tag="ye",
                                 bufs=3)
                    nc.vector.tensor_copy(ye, holder[0])
                    nc.sync.dma_start(
                        y.ap()[t * P:(t + 1) * P, n * 512:(n + 1) * 512], ye)
                FQ.append(ev)

        def pump(n):
            while n > 0 and FQ:
                item = FQ.popleft()
                if callable(item):
                    item()
                    n -= 1

        def drain_to(tag):
            while FQ:
                item = FQ.popleft()
                if callable(item):
                    item()
                elif item == tag:
                    return

        # queue order = deadline order. v/qk chunks for qc+1 and the
        # outproj of qc are queued as qc's pairs finish, so outproj fill
        # material is available mid-attention instead of piling at the
        # tail (an empty fill queue leaves the PE micro-idling on the
        # ACT WAR wait each pair-step, which sets off HAM re-throttle
        # oscillation).
        def queue_vqk(qc):
            for tb in range(4 * qc, 4 * qc + 4):
                v_chunk(tb)
            for m in range(NMQ):
                qk_chunk("q", m, qc)
                qk_chunk("k", m, qc)
            FQ.append(f"qc{qc}")

        for m in range(1, NMQ):
            qk_chunk("q", m, 0)
            qk_chunk("k", m, 0)
            FQ.append(f"m{m}c0")
        queue_vqk(1)

        # ================= attention =====================================
        def attention_pair(qc, hp, fpump=2):
            c0 = qc * 512
            vle, vlo = 65 * (2 * hp), 65 * (2 * hp + 1)
            po_e = pp.tile([P, 512], FP, name=f"poe{qc}{hp}", tag="po",
                           bufs=2)
            po_o = pp.tile([P, 512], FP, name=f"poo{qc}{hp}", tag="po",
                           bufs=2)
            jmax = 4 * qc + 3

            def flush_av(pend):
                spans, es_e, es_o = pend
                for j, lo, d, w in spans:
                    nc.tensor.matmul(
                        po_e[0:65, lo:lo + w], vsb[j][:, vle:vle + 65],
                        es_e[:, d:d + w],
                        start=(j == 0), stop=(j == jmax))
                for j, lo, d, w in spans:
                    nc.tensor.matmul(
                        po_o[0:65, lo:lo + w], vsb[j][:, vlo:vlo + 65],
                        es_o[:, d:d + w],
                        start=(j == 0), stop=(j == jmax))

            # two-deep software pipeline: AV consumes the exp from two
            # steps back, so the in-order tensor queue never parks on ACT.
            pends = deque()
            for pr in range(2 * qc + 2):
                spans = []
                dst = 0
                for i in range(2):
                    j = 2 * pr + i
                    r = j - 4 * qc
                    lo = 0 if r < 0 else 128 * r
                    w = 512 - lo
                    dst = max(dst, i * 512 if r < 1 else 0)
                    spans.append((j, lo, dst, w))
                    dst += w
                pump(2)
                if len(pends) >= 2:
                    flush_av(pends.popleft())
                pump(max(0, PUMP[qc] - 2))
                ps_e = pp.tile([P, 1024], FP, name=f"pse{qc}{hp}{pr}",
                               tag="wide", bufs=2)
                ps_o = pp.tile([P, 1024], FP, name=f"pso{qc}{hp}{pr}",
                               tag="wide", bufs=2)
                # row-tiled: even head on PE rows 0:64, odd on 64:128.
                # Interleave e/o per span so consecutive matmuls sit on
                # different row groups: the odd ldweights pulls ahead and
                # both streams run concurrently on the PE.
                for j, lo, d, w in spans:
                    nc.tensor.matmul(
                        ps_e[:, d:d + w], kT[hp][0:64, j * P:(j + 1) * P],
                        qT[hp][0:64, c0 + lo:c0 + 512],
                        start=True, stop=True)
                    nc.tensor.matmul(
                        ps_o[:, d:d + w], kT[hp][64:128, j * P:(j + 1) * P],
                        qT[hp][64:128, c0 + lo:c0 + 512],
                        start=True, stop=True)
                es_e = ep.tile([P, 1024], BF, name=f"ese{qc}{hp}{pr}",
                               tag="es", bufs=8)
                es_o = ep.tile([P, 1024], BF, name=f"eso{qc}{hp}{pr}",
                               tag="es", bufs=8)
                e0 = spans[0][2]
                e1 = spans[1][2] + spans[1][3]
                nc.scalar.activation(es_e[:, e0:e1], ps_e[:, e0:e1], EXP,
                                     scale=SCALE)
                nc.scalar.activation(es_o[:, e0:e1], ps_o[:, e0:e1], EXP,
                                     scale=SCALE)
                for j, lo, d, w in spans:
                    if j - 4 * qc >= 0:
                        nc.vector.tensor_mul(
                            es_e[:, d:d + 128], es_e[:, d:d + 128], trit)
                        nc.vector.tensor_mul(
                            es_o[:, d:d + 128], es_o[:, d:d + 128], trit)
                pends.append((spans, es_e, es_o))
            while pends:
                pump(fpump)
                flush_av(pends.popleft())

            # denominator row 64 -> 1/den -> broadcast -> normalize the
            # numerator straight out of PSUM into aT. Fill matmuls are
            # pumped between the steps so the PE never idles under this
            # latency chain (it is fully exposed on the final pair).
            den_e = ep.tile([1, 512], FP, name=f"dne{qc}{hp}", tag="dn",
                            bufs=4)
            den_o = ep.tile([1, 512], FP, name=f"dno{qc}{hp}", tag="dn",
                            bufs=4)
            nc.vector.tensor_copy(den_e, po_e[64:65, :])
            nc.vector.tensor_copy(den_o, po_o[64:65, :])
            pump(2)
            rs = ep.tile([1, 1024], FP, name=f"rs{qc}{hp}", tag="rs",
                         bufs=4)
            nc.vector.reciprocal_approx_fast(out=rs[:, 0:512], in_=den_e)
            nc.vector.reciprocal_approx_fast(out=rs[:, 512:1024], in_=den_o)
            pump(2)
            bcs = ep.tile([64, 1024], FP, name=f"bc{qc}{hp}", tag="bcs",
                          bufs=4)
            nc.gpsimd.partition_broadcast(bcs, rs)
            pump(2)
            nc.vector.tensor_mul(aT[hp][0:64, c0:c0 + 512], po_e[0:64, :],
                                 bcs[:, 0:512])
            nc.vector.tensor_mul(aT[hp][64:128, c0:c0 + 512], po_o[0:64, :],
                                 bcs[:, 512:1024])

        for qc in range(NQC):
            if qc > 0:
                drain_to(f"qc{qc}")
            for hp in range(NHP):
                if qc == 0 and hp < NHP - 1:
                    # prefetch-drain the NEXT pair's q/k chunks so their
                    # DVE evictions complete during this pair's attention
                    drain_to(f"m{hp + 1}c0")
                attention_pair(qc, hp, fpump=2)
            for t in range(4 * qc, 4 * qc + 4):
                outproj_tile(t)
            if qc + 2 <= NQC - 1:
                queue_vqk(qc + 2)

        drain_to(None)

    nc.compile()
    _NC_CACHE["nc"] = nc
    return nc


def kernel(x, W_qkv, b_qkv, W_out, b_out):
    global _LAST_IN_MAPS
    x = np.asarray(x, dtype=np.float32)
    W_qkv = np.asarray(W_qkv, dtype=np.float32)
    b_qkv = np.asarray(b_qkv, dtype=np.float32)
    W_out = np.asarray(W_out, dtype=np.float32)
    b_out = np.asarray(b_out, dtype=np.float32)
    import ml_dtypes

    bf16 = ml_dtypes.bfloat16
    tri = np.triu(np.ones((P, P), dtype=np.float32)).astype(bf16)
    in_maps = []
    for c in range(NCORES):
        b, hg = c // 2, c % 2
        cols = slice(hg * QK, (hg + 1) * QK)
        wq = W_qkv[:, 0 * C:1 * C][:, cols]
        wk = W_qkv[:, 1 * C:2 * C][:, cols]
        wv = W_qkv[:, 2 * C:3 * C][:, cols]
        in_maps.append({
            "xT": np.ascontiguousarray(x[b].T).astype(bf16),
            "Wq": np.ascontiguousarray(
                wq.reshape(NKT, P, NMQ, P).transpose(2, 1, 0, 3)
                .reshape(NMQ, P, NKT * P)).astype(bf16),
            "Wk": np.ascontiguousarray(
                wk.reshape(NKT, P, NMQ, P).transpose(2, 1, 0, 3)
                .reshape(NMQ, P, NKT * P)).astype(bf16),
            "Wv": np.ascontiguousarray(
                wv.reshape(NKT, P, QK).transpose(1, 0, 2)
                .reshape(P, NKT * QK)).astype(bf16),
            "bq": np.ascontiguousarray(b_qkv[0 * C:1 * C][cols, None]),
            "bk": np.ascontiguousarray(b_qkv[1 * C:2 * C][cols, None]),
            "Wo": np.ascontiguousarray(W_out[hg * QK:(hg + 1) * QK, :]).astype(bf16),
            "tri": tri,
        })
    _LAST_IN_MAPS = in_maps
    nc = build_nc()
    res = run_bass_kernel_spmd(nc, in_maps, core_ids=list(range(NCORES)))
    # v-bias and output bias are affine in the output: softmax rows sum to 1.
    extra = b_qkv[2 * C:3 * C] @ W_out + b_out
    out = np.empty((B, T, C), dtype=np.float32)
    for b in range(B):
        out[b] = (res.results[2 * b]["y"].astype(np.float32)
                  + res.results[2 * b + 1]["y"].astype(np.float32) + extra)
    return out

